# revision 110
# baseline (speedup 1.0000x reference)
"""BitNet attention layer on 8 Trainium2 NeuronCores (156.6us -> 125.2us).

Tensor-parallel over heads: core i owns heads {2i, 2i+1}. Each core:
  - computes q^T,k^T (feature-major) + v (natural) for its heads via fp8
    DoubleRow matmuls (0.5 cyc/row, paired h-chunks) against host-split
    x = x_hi + x_res (both fp8e4m3) and the exactly-fp8 ternary W^T slices;
    hi+res accumulation in one psum chain recovers ~bf16 accuracy at 2.6x
    the bf16 matmul rate (BitNet ternary weights are exact in fp8e4m3)
  - RoPE on q^T/k^T via an ACT psum->sbuf pre-copy + 4 SBUF-only DVE muls
    (rotate-half sign folded into a host-swapped sin table so both SBUF
    operands of each mul share a base partition)
  - causal attention with transposed scores S^T[k,q] in bf16; diagonal
    128-blocks (emitted first, masked by Pool tri-muls with a pipeline
    step of slack) use triangular free-dim slices so fully-masked columns
    are never computed; full-chunk pairs share a 2-bank psum tile and one
    free-1024 exp; softmax denominator via DVE chunk pre-sum + one
    ones-matmul partition reduce per (tile, head), whose bp scratch shares
    the ctx psum pool rotation so scores tiles are never held hostage
  - ctx normalize -> bf16 (DVE), fp8 hi (ACT copy) + fp8 res (Pool sub)
  - o_proj partial over its 256 ctx features via fp8 DoubleRow (head-paired
    contraction, hi then res chains) -> fp16 partial [2048, 2048]
Host sums the 8 partials.

Schedule notes: scores-chunk units are zipper-interleaved with o_proj
blocks (a stalled scores matmul can only bypass 4-deep in the PE queue);
t0/t1 scores ride the projection phase; st1's x_hi prefetch jumps the
table DMAs; the last tile's ctx runs in qi-halves for both heads with
o_proj blocks slotted between; psum->sbuf output copies round-robin
ACT/DVE (GPSIMD cannot touch PSUM), Pool takes SBUF-only elementwise
work off the DVE/ACT critical paths.
"""
import os
import sys

import numpy as np

try:
    import concourse.bass as bass
except ImportError:
    sys.path.insert(0, "/opt/trn_rl_repo")
    import concourse.bass as bass

import concourse.mybir as mybir
import concourse.tile as tile
from concourse import bacc
from concourse.bass_utils import run_bass_kernel_spmd

F32 = mybir.dt.float32
F32R = mybir.dt.float32r
F16 = mybir.dt.float16
F8 = mybir.dt.float8e4
BF16 = mybir.dt.bfloat16
DR = mybir.MatmulPerfMode.DoubleRow

S = 2048          # sequence length
H = 2048          # hidden
D = 128           # head dim
NCORES = 8
HPC = 2           # heads per core
OC = 3 * HPC * D  # 768 per-core projection output features (q|k|v)
ST = 512          # seq tile for projection rhs / attention qi tile
NST = S // ST     # 4
HC = H // 128     # 16 h-chunks
HG = 2            # h-chunk group size (DMA granularity)
NG = HC // HG     # 8 groups
ROPE_BASE = 10000.0

_built = None
_PHASES = os.environ.get("KPH", "ABC")


def _build(timing=False):
    nc = bacc.Bacc("TRN2", target_bir_lowering=False, debug=False,
                   dynamic_dma_scratch_size=4096)

    if timing:
        # timing variant: identical device work, but big tensors live in
        # internal DRAM (garbage data) so per-call host<->device transfer is
        # tiny and wall-clock deltas measure the NEFF itself.
        xh_d = nc.dram_tensor("xh_i", [H, S], F8)
        xr_d = nc.dram_tensor("xr_i", [H, S], F8)
        wt_d = nc.dram_tensor("wt_i", [H, OC], F8)
        wot_d = nc.dram_tensor("wot_i", [HPC * D, H], F8)
        cos_d = nc.dram_tensor("cost_i", [D, S], BF16)
        sin_d = nc.dram_tensor("sins_i", [D, S], BF16)
        tri_d = nc.dram_tensor("tri_i", [128, 896], BF16)
        out_d = nc.dram_tensor("out_i", [S, H], F16)
        out_x = nc.declare_dram_parameter("out", [128, H], F16, isOutput=True)
    else:
        xh_d = nc.declare_dram_parameter("xh", [H, S], F8, isOutput=False)
        xr_d = nc.declare_dram_parameter("xr", [H, S], F8, isOutput=False)
        wt_d = nc.declare_dram_parameter("wt", [H, OC], F8, isOutput=False)
        wot_d = nc.declare_dram_parameter("wot", [HPC * D, H], F8,
                                          isOutput=False)
        cos_d = nc.declare_dram_parameter("cost", [D, S], BF16, isOutput=False)
        sin_d = nc.declare_dram_parameter("sins", [D, S], BF16, isOutput=False)
        tri_d = nc.declare_dram_parameter("tri", [128, 896], BF16,
                                          isOutput=False)
        out_d = nc.declare_dram_parameter("out", [S, H], F16, isOutput=True)
    osq_d = nc.declare_dram_parameter("osq", [128, 128], BF16, isOutput=False)

    # exp scale (s_p^2/sqrt(D)) and output scale (s_p*s_o) are runtime values;
    # pass them as tiny per-partition inputs instead of baking into the NEFF.
    esc_d = nc.declare_dram_parameter("esc", [128, 1], F32, isOutput=False)
    osc_d = nc.declare_dram_parameter("osc", [128, 1], F32, isOutput=False)

    with tile.TileContext(nc) as tc, nc.allow_low_precision(
        reason="bf16 matmul operands / probs; validated 3.3e-3 rel err"
    ):
        with tc.tile_pool(name="const", bufs=1) as cpool, \
             tc.tile_pool(name="qkv", bufs=1) as qpool, \
             tc.tile_pool(name="ctx", bufs=1) as xpool, \
             tc.tile_pool(name="wo", bufs=1) as wopool, \
             tc.tile_pool(name="ob", bufs=5) as opool, \
             tc.tile_pool(name="pt", bufs=2) as ptpool, \
             tc.tile_pool(name="rden", bufs=2) as dpool, \
             tc.tile_pool(name="ptsum", bufs=4) as spool:
            cost = cpool.tile([D, S], BF16)
            sins = cpool.tile([D, S], BF16)
            tri = cpool.tile([128, 896], BF16)
            osq = cpool.tile([128, 128], BF16)
            esc = cpool.tile([128, 1], F32)
            osc = cpool.tile([128, 1], F32)
            wot = wopool.tile([128, HPC, H], F8)

            # persistent per-head tensors, tiled per seq-tile for fine deps
            qk = [[qpool.tile([D, ST], BF16, name=f"qk{oc}_{st}")
                   for st in range(NST)] for oc in range(4)]
            v_sb = [qpool.tile([128, ST // 128, HPC * D], BF16, name=f"v{st}")
                    for st in range(NST)]
            # ctx in fp8 hi+res, heads adjacent on dim1 so o_proj DoubleRow
            # can contract (d, head) pairs in one AP
            chi = [xpool.tile([D, HPC, ST], F8, name=f"chi{t}")
                   for t in range(NST)]
            cres = [xpool.tile([D, HPC, ST], F8, name=f"cres{t}")
                    for t in range(NST)]

            def chunk_order(t):
                """Diagonal chunks first so their Pool tri-muls are long done
                when the ctx chain (which consumes diag chunks last-minute
                otherwise) runs; full chunks follow in pairs."""
                nkj = 4 * (t + 1)
                nfull = 4 * t
                return list(range(nfull, nkj)) + list(range(nfull)), nfull

            def attn_scores_units(t, h, pool, pair=False):
                """scores S^T[kj, qi] per 128-chunk; diagonal chunks first
                (their Pool tri-muls get a full pipeline step of slack), then
                full chunks. With pair=True, full chunk pairs share a 2-bank
                psum tile and a single free-1024 exp. Returns
                (pt, ptsum, [unit closures]) — each closure emits one
                chunk(-pair)'s matmuls+exp+mask+pre-sum so the caller can
                interleave them with other PE work."""
                order, nfull = chunk_order(t)
                nkj = 4 * (t + 1)
                pt = ptpool.tile([128, nkj, ST], BF16, name=f"pt{h}")
                ptsum = spool.tile([128, ST], BF16, name="ptsum")
                ptsum2 = None
                state = {"first": True, "first2": True}

                def kc(j):
                    return qk[2 + h][j // 4][:, (j % 4) * 128:
                                             (j % 4 + 1) * 128]

                def presum(jj):
                    di2 = jj - nfull
                    lo2 = 128 * di2 if di2 >= 0 else 0
                    if di2 >= 0 or ptsum2 is None:
                        # diag chunks: DVE accumulator (latency-critical)
                        if state["first"]:
                            nc.vector.tensor_copy(ptsum[:, lo2:],
                                                  pt[:, jj, lo2:])
                            state["first"] = False
                        else:
                            nc.vector.tensor_add(ptsum[:, lo2:],
                                                 ptsum[:, lo2:],
                                                 pt[:, jj, lo2:])
                    else:
                        # full chunks: Pool accumulator, merged by a second
                        # ones-matmul in attn_ctx
                        if state["first2"]:
                            nc.gpsimd.tensor_copy(ptsum2[:], pt[:, jj, :])
                            state["first2"] = False
                        else:
                            nc.gpsimd.tensor_add(ptsum2[:], ptsum2[:],
                                                 pt[:, jj, :])

                def unit_pair(j):
                    def emit():
                        sp2 = pool.tile([128, 2, ST], F32, name="sp")
                        for u in range(2):
                            nc.tensor.matmul(sp2[:, u, :], kc(j + u),
                                             qk[h][t][:], start=True,
                                             stop=True)
                        nc.scalar.activation(
                            pt[:, j:j + 2, :], sp2[:],
                            mybir.ActivationFunctionType.Exp,
                            bias=0.0, scale=esc[:])
                        presum(j)
                        presum(j + 1)
                    return emit

                def unit_single(j):
                    def emit():
                        di = j - nfull
                        lo = 128 * di if di >= 0 else 0
                        sp2 = pool.tile([128, 2, ST], F32, name="sp") \
                            if pair else pool.tile([128, ST], F32, name="sp")
                        sp = sp2[:, 0, :] if pair else sp2[:]
                        nc.tensor.matmul(
                            sp[:, lo:], kc(j), qk[h][t][:, lo:],
                            start=True, stop=True)
                        # probs (unnormalized): exp(esc * scores)
                        nc.scalar.activation(
                            pt[:, j, lo:], sp[:, lo:],
                            mybir.ActivationFunctionType.Exp,
                            bias=0.0, scale=esc[:])
                        if di >= 0:  # diagonal block: tril mask (Pool;
                            # DVE for the last tile where the ctx chain
                            # consumes it with no pipeline step of slack)
                            eng = nc.vector if t == NST - 1 else nc.gpsimd
                            eng.tensor_mul(
                                pt[:, j, lo:], pt[:, j, lo:],
                                tri[:, 384:896 - lo])
                        presum(j)
                    return emit

                units = []
                oi = 0
                while oi < len(order):
                    j = order[oi]
                    di = j - nfull       # >= 0 on diagonal chunks
                    if pair and di < 0 and oi + 1 < len(order):
                        units.append(unit_pair(j))
                        oi += 2
                    else:
                        units.append(unit_single(j))
                        oi += 1
                return pt, (ptsum, ptsum2), units

            def attn_scores(t, h, pool, pair=False):
                pt, ptsums, units = attn_scores_units(t, h, pool, pair)
                for u in units:
                    u()
                return pt, ptsums

            _s0 = []

            # ---------------- Phase A: qkv projection + RoPE ----------------
            if "A" in _PHASES:
             with tc.tile_pool(name="wt", bufs=1) as wpool, \
                 tc.tile_pool(name="xt", bufs=2) as xtpool, \
                 tc.tile_pool(name="ropet", bufs=3) as rpool, \
                 tc.tile_pool(name="psA", bufs=4, space="PSUM") as psA, \
                 tc.tile_pool(name="psSa", bufs=2, space="PSUM") as psSa, \
                 tc.tile_pool(name="psV", bufs=2, space="PSUM") as psV:
                wt = [wpool.tile([128, HG, OC], F8, name=f"wt{g}")
                      for g in range(NG)]
                xh0 = [xtpool.tile([128, HG, ST], F8, name=f"xh{g}")
                       for g in range(NG)]
                xr0 = [xtpool.tile([128, HG, ST], F8, name=f"xr{g}")
                       for g in range(NG)]
                # warm up the PE p-state immediately on a memset tile (no
                # DMA dependency), so the tensor clock is ramping while the
                # wt/xt stream arrives
                wmz = rpool.tile([128, 128], BF16, name="wmz")
                nc.any.memset(wmz[:], 0)
                wps = psA.tile([128, ST], F32, name="ps")
                for _ in range(6):
                    nc.tensor.matmul(wps[:, 0:128], wmz[:], wmz[:],
                                     start=True, stop=True)
                # critical startup stream: weight group (SP queue) with the
                # matching x_hi group issued from the ACT queue so the two
                # HWDGE descriptor generators run in parallel
                for g in range(NG):
                    nc.sync.dma_start(
                        wt[g][:],
                        wt_d[g * HG * 128:(g + 1) * HG * 128].rearrange(
                            "(ho hp) o -> hp ho o", hp=128))
                    nc.scalar.dma_start(
                        xh0[g][:],
                        xh_d[g * HG * 128:(g + 1) * HG * 128, 0:ST].rearrange(
                            "(ho hp) s -> hp ho s", hp=128))
                for g in range(NG):
                    nc.sync.dma_start(
                        xr0[g][:],
                        xr_d[g * HG * 128:(g + 1) * HG * 128, 0:ST].rearrange(
                            "(ho hp) s -> hp ho s", hp=128))
                # rope tables next (needed at st0's end, ACT queue); heavier
                # non-critical constants are deferred behind st2's xt stream
                # st1's x_hi prefetch jumps the table queue: its hi chains
                # are the next PE consumers after st0
                xhw1 = xtpool.tile([128, HC, ST], F8, name="xhw")
                nc.sync.dma_start(
                    xhw1[:],
                    xh_d[:, ST:2 * ST].rearrange("(ho hp) s -> hp ho s",
                                                 hp=128))
                nc.sync.dma_start(cost[:], cos_d[:])
                nc.sync.dma_start(sins[:], sin_d[:])
                xrw1 = xtpool.tile([128, HC, ST], F8, name="xrw")
                nc.sync.dma_start(
                    xrw1[:],
                    xr_d[:, ST:2 * ST].rearrange("(ho hp) s -> hp ho s",
                                                 hp=128))
                nc.sync.dma_start(tri[:], tri_d[:])
                nc.sync.dma_start(osq[:], osq_d[:])

                def rope(dst, ps, ssl, eng=None):
                    # ACT drains psum once; all 4 elementwise ops then run at
                    # SBUF bf16 rates (no psum access penalty). eng picks the
                    # elementwise engine (DVE default; Pool for the last tile
                    # whose ropes would otherwise sit in DVE's B-phase queue)
                    eng = eng or nc.vector
                    q8 = rpool.tile([128, ST], BF16, name="q8")
                    nc.scalar.activation(
                        q8[:], ps[:], mybir.ActivationFunctionType.Copy,
                        bias=0.0, scale=1.0)
                    # sins table halves are host-swapped so each mul's two
                    # SBUF inputs share a base partition (walrus constraint)
                    t2 = rpool.tile([128, ST], BF16, name="t2")
                    eng.tensor_mul(t2[0:64, :], q8[64:128, :],
                                   sins[64:128, ssl])
                    eng.tensor_mul(t2[64:128, :], q8[0:64, :],
                                   sins[0:64, ssl])
                    eng.tensor_mul(dst[:], q8[:], cost[:, ssl])
                    eng.tensor_add(dst[:], dst[:], t2[:])

                def v_chain(st, xh, xr, sc):
                    ps = psV.tile([128, HPC * D], F32)
                    for rp, xx in enumerate((xh, xr)):
                        for g in range(NG):
                            nc.tensor.matmul(
                                ps[:],
                                xx[g][:, :, sc * 128:(sc + 1) * 128],
                                wt[g][:, :, 4 * 128:],
                                start=(rp == 0 and g == 0),
                                stop=(rp == 1 and g == NG - 1),
                                perf_mode=DR)
                    # psum drain on ACT: DVE's in-order queue is full of rope
                    # ops, and stalling this copy stalls psV recycling (PE)
                    nc.scalar.activation(
                        v_sb[st][:, sc, :], ps[:],
                        mybir.ActivationFunctionType.Copy,
                        bias=0.0, scale=1.0)

                def v_pass(st, xh, xr):
                    for sc in range(ST // 128):
                        v_chain(st, xh, xr, sc)

                # st0: group-major over x_hi so the PE consumes groups as
                # they arrive, then the x_res closing pass
                psA0 = [psA.tile([128, ST], F32, name="ps") for oc in range(4)]
                for rp, xx in enumerate((xh0, xr0)):
                    for g in range(NG):
                        for oc in range(4):
                            nc.tensor.matmul(
                                psA0[oc][:],
                                wt[g][:, :, oc * 128:(oc + 1) * 128],
                                xx[g][:, :, :],
                                start=(rp == 0 and g == 0),
                                stop=(rp == 1 and g == NG - 1),
                                perf_mode=DR)
                for oc in (2, 0, 3, 1):
                    rope(qk[oc][0], psA0[oc], slice(0, ST))
                v_pass(0, xh0, xr0)

                # st1..3: oc-major chains, xt double-buffered; prefetched a
                # full tile ahead so per-group DMA granularity buys nothing —
                # one descriptor-gen per tensor keeps the HWDGE queues clear
                for st in range(1, NST):
                    ssl = slice(st * ST, (st + 1) * ST)
                    if st == 1:
                        xhw, xrw = xhw1, xrw1
                    else:
                        xhw = xtpool.tile([128, HC, ST], F8, name="xhw")
                        xrw = xtpool.tile([128, HC, ST], F8, name="xrw")
                        nc.sync.dma_start(
                            xhw[:],
                            xh_d[:, ssl].rearrange("(ho hp) s -> hp ho s",
                                                   hp=128))
                        nc.sync.dma_start(
                            xrw[:],
                            xr_d[:, ssl].rearrange("(ho hp) s -> hp ho s",
                                                   hp=128))
                    xh = [xhw[:, g * HG:(g + 1) * HG, :] for g in range(NG)]
                    xr = [xrw[:, g * HG:(g + 1) * HG, :] for g in range(NG)]
                    if st == 1:
                        nc.sync.dma_start(esc[:], esc_d[:])
                        nc.sync.dma_start(osc[:], osc_d[:])
                    if st == 2:
                        nc.sync.dma_start(
                            wot[:],
                            wot_d.rearrange("(co cp) o -> cp co o", cp=128))
                    if st == NST - 1:
                        # last tile: v first, so the B-phase PE stream follows
                        # the qk chains directly and the final ropes (DVE)
                        # overlap B's first ctx chain
                        v_pass(st, xh, xr)
                    for oc in range(4):
                        ps = psA.tile([128, ST], F32, name="ps")
                        for rp, xx in enumerate((xh, xr)):
                            for g in range(NG):
                                nc.tensor.matmul(
                                    ps[:],
                                    wt[g][:, :, oc * 128:(oc + 1) * 128],
                                    xx[g][:, :, :],
                                    start=(rp == 0 and g == 0),
                                    stop=(rp == 1 and g == NG - 1),
                                    perf_mode=DR)
                        rope(qk[oc][st], ps, ssl)
                    if st != NST - 1:
                        v_pass(st, xh, xr)
                    if "B" in _PHASES:
                        # early-tile scores ride the projection: psum banks
                        # are free, ACT is otherwise idle here, and the qk
                        # tiles they read have been ready since st-1
                        if st == 1:
                            _s0.append(attn_scores(0, 0, psSa))
                            _s0.append(attn_scores(0, 1, psSa))
                        elif st == 2:
                            _s0.append(attn_scores(1, 0, psSa))
                            _s0.append(attn_scores(1, 1, psSa))


            # ---------- Phase B+C: attention + o_proj, interleaved ----------
            def copy_scaled(engine, dst, src):
                if engine == "act":
                    nc.scalar.activation(
                        dst, src, mybir.ActivationFunctionType.Copy,
                        bias=0.0, scale=osc[:])
                elif engine == "dve":
                    nc.vector.tensor_scalar_mul(dst, src, osc[:])
                else:
                    nc.gpsimd.tensor_scalar_mul(dst, src, osc[:])

            def oproj_mms(po_ap, t, sc, ot):
                """One [128, ST] o_proj output chain: fp8 DoubleRow over the
                (d, head) contraction, hi chain then res chain."""
                for ci, cc_t in enumerate((chi[t], cres[t])):
                    nc.tensor.matmul(
                        po_ap,
                        cc_t[:, :, (sc % 4) * 128:(sc % 4 + 1) * 128],
                        wot[:, :, ot * ST:(ot + 1) * ST],
                        start=(ci == 0), stop=(ci == 1),
                        perf_mode=DR)

            def oproj_ob(t, sc, half, psO_, engines, ei, fused):
                """One [128, H/2] output block: two psum chains, head-0 matmul
                first in each chain so the PE can proceed while head-1's ctx
                normalize drains; psum->sbuf copies round-robin `engines`.
                fused: both chains in one 2-bank psum tile, one [128,1024]
                copy."""
                ob = opool.tile([128, H // 2], F16)
                if fused:
                    po = psO_.tile([128, 2, ST], F32, name="po")
                    for oth in range(2):
                        oproj_mms(po[:, oth, :], t, sc, half * 2 + oth)
                    if fused == "tail":
                        # final block: parallel ACT+DVE half copies and two
                        # half DMAs so the drain after the last matmul is
                        # as short as possible
                        for oth in range(2):
                            copy_scaled(["act", "dve"][oth],
                                        ob[:, oth * ST:(oth + 1) * ST],
                                        po[:, oth, :])
                            nc.sync.dma_start(
                                out_d[sc * 128:(sc + 1) * 128,
                                      half * (H // 2) + oth * ST:
                                      half * (H // 2) + (oth + 1) * ST],
                                ob[:, oth * ST:(oth + 1) * ST])
                        return
                    copy_scaled(engines[ei % len(engines)], ob[:], po[:])
                else:
                    for oth in range(2):
                        po1 = psO_.tile([128, ST], F32, name="po")
                        oproj_mms(po1[:], t, sc, half * 2 + oth)
                        copy_scaled(engines[(ei + oth) % len(engines)],
                                    ob[:, oth * ST:(oth + 1) * ST], po1[:])
                nc.sync.dma_start(
                    out_d[sc * 128:(sc + 1) * 128,
                          half * (H // 2):(half + 1) * (H // 2)], ob[:])

            def oproj_blocks(t, psO_, engines, fused=False, skip=0,
                             tail=False):
                blocks = []
                ei = 0
                for sc in range(4 * t, 4 * t + 4):
                    for half in range(2):
                        if ei >= skip:
                            f = ("tail" if (tail and ei == 7) else fused)
                            blocks.append(
                                lambda sc=sc, half=half, ei=ei, f=f:
                                oproj_ob(t, sc, half, psO_, engines, ei, f))
                        ei += 1
                return blocks

            def oproj(t, psO_, engines, fused=False, skip=0, tail=False):
                for b in oproj_blocks(t, psO_, engines, fused, skip, tail):
                    b()

            def zip_emit(units, blocks):
                """Interleave scores units with o_proj blocks so a stalled
                scores matmul (waiting on ACT to free its psum pair) never
                head-of-line-blocks the o_proj stream in the PE queue."""
                ui = 0
                lead = min(1, len(units))
                for u in units[:lead]:
                    u()
                ui = lead
                for bi, b in enumerate(blocks):
                    tgt = lead + ((bi + 1) * (len(units) - lead)) // len(blocks)
                    while ui < tgt:
                        units[ui]()
                        ui += 1
                    b()
                while ui < len(units):
                    units[ui]()
                    ui += 1

            if "B" in _PHASES:
             with tc.tile_pool(name="psO", bufs=2, space="PSUM") as psO, \
                 tc.tile_pool(name="psSb", bufs=2, space="PSUM") as psSb, \
                 tc.tile_pool(name="ctmp", bufs=2) as cpool2, \
                 tc.tile_pool(name="psC", bufs=2, space="PSUM") as psC:
                def attn_ctx(t, h, pt, ptsums, halves=None):
                    """ctx^T[d, qi] accumulate over kj; the denominator
                    ones-matmul + reciprocal are emitted mid-chain so rbp is
                    ready before cp completes and the final normalize costs a
                    single DVE mul. `halves` restricts the qi range so the
                    last tile can interleave both heads' halves with o_proj
                    (half A's chi feeds o_proj while half B still streams)."""
                    ptsum, ptsum2 = ptsums
                    if halves is None:
                        halves = [(0, ST)]
                    rbp = dpool.tile([128, ST], F32, name=f"rbp{h}")
                    order, nfull = chunk_order(t)
                    for qa, qb in halves:
                        # bp shares the cp rotation (released right after the
                        # reciprocal, well before this half's cp) so scores
                        # pair tiles in psSb are never held hostage by ctx
                        cp = psC.tile([128, ST], F32, name="cp")
                        bp = psC.tile([128, ST], F32, name="cp")
                        chunks = [j for j in order
                                  if (128 * (j - nfull)
                                      if j >= nfull else 0) < qb]
                        for ji, j in enumerate(chunks):
                            di = j - nfull
                            lo = max(128 * di if di >= 0 else 0, qa)
                            nc.tensor.matmul(
                                cp[:, lo:qb],
                                v_sb[j // 4][:, j % 4, h * D:(h + 1) * D],
                                pt[:, j, lo:qb],
                                start=(ji == 0), stop=(ji == len(chunks) - 1))
                            if ji == len(chunks) - 2:
                                # pre-sums complete by now (trail the exps)
                                nc.tensor.matmul(
                                    bp[:, qa:qb], osq[:], ptsum[:, qa:qb],
                                    start=True, stop=(ptsum2 is None))
                                if ptsum2 is not None:
                                    nc.tensor.matmul(
                                        bp[:, qa:qb], osq[:],
                                        ptsum2[:, qa:qb],
                                        start=False, stop=True)
                                nc.vector.reciprocal(rbp[:, qa:qb],
                                                     bp[:, qa:qb])
                        # normalize to bf16 (DVE), fp8 hi via ACT copy,
                        # residual on Pool
                        ct = cpool2.tile([128, ST], BF16, name="ct")
                        nc.vector.tensor_mul(ct[:, qa:qb],
                                             cp[:, qa:qb], rbp[:, qa:qb])
                        nc.scalar.activation(
                            chi[t][:, h, qa:qb], ct[:, qa:qb],
                            mybir.ActivationFunctionType.Copy,
                            bias=0.0, scale=1.0)
                        nc.gpsimd.tensor_sub(cres[t][:, h, qa:qb],
                                             ct[:, qa:qb],
                                             chi[t][:, h, qa:qb])

                # software pipeline: scores(t+1, 0) is emitted before
                # oproj(t) so the ACT engine's exp stream stays dense while
                # o_proj drains
                attn_ctx(0, 0, *_s0[0])
                attn_ctx(0, 1, *_s0[1])
                s20pt, s20sum, s20u = attn_scores_units(2, 0, psSb, pair=True)
                if "C" in _PHASES:
                    zip_emit(s20u, oproj_blocks(0, psO, ["act", "dve"]))
                else:
                    for u in s20u:
                        u()
                attn_ctx(1, 0, *_s0[2])
                attn_ctx(1, 1, *_s0[3])
                s21pt, s21sum, s21u = attn_scores_units(2, 1, psSb, pair=True)
                if "C" in _PHASES:
                    zip_emit(s21u, oproj_blocks(1, psO, ["act", "dve"]))
                else:
                    for u in s21u:
                        u()
                attn_ctx(2, 0, s20pt, s20sum)
                attn_ctx(2, 1, s21pt, s21sum)
                s30pt, s30sum, s30u = attn_scores_units(3, 0, psSb, pair=True)
                s31pt, s31sum, s31u = attn_scores_units(3, 1, psSb, pair=True)
                if "C" in _PHASES:
                    for u in s30u:
                        u()
                    zip_emit(s31u, oproj_blocks(2, psO, ["act", "dve"]))
                else:
                    for u in s30u + s31u:
                        u()
                s30 = (s30pt, s30sum)
                s31 = (s31pt, s31sum)
                # last tile: both heads' ctx split in qi-halves; half A's
                # chi/cres feed the first o_proj blocks while half B streams
                attn_ctx(3, 0, *s30, halves=[(0, 256)])
                attn_ctx(3, 1, *s31, halves=[(0, 256)])
                attn_ctx(3, 0, *s30, halves=[(256, ST)])
                if "C" in _PHASES:
                    t = NST - 1
                    # blocks over qi chunks 12-13 need only half A of chi
                    oproj_ob(t, 4 * t, 0, psO, ["act", "dve"], 0,
                             fused=False)
                    oproj_ob(t, 4 * t, 1, psO, ["act", "dve"], 1,
                             fused=False)
                    oproj_ob(t, 4 * t + 1, 0, psO, ["dve", "act"], 0,
                             fused=False)
                attn_ctx(3, 1, *s31, halves=[(256, ST)])
                if "C" in _PHASES:
                    oproj_ob(t, 4 * t + 1, 1, psO, ["dve", "act"], 1,
                             fused=False)

            # rest of the last tile's o_proj: nothing left to interleave, so
            # deep fused psum buffering, copies ending on the faster ACT
            if "B" in _PHASES and "C" in _PHASES:
                with tc.tile_pool(name="psO3", bufs=3, space="PSUM") as psO3:
                    oproj(NST - 1, psO3, ["act", "dve"], fused=True,
                          skip=4, tail=True)

            if timing:
                nc.sync.dma_start(out_x[:], out_d[S - 128:, :])

    nc.compile()
    return nc


def _host_prep(hidden_states, w_proj, w_o):
    import ml_dtypes
    x = np.asarray(hidden_states, dtype=np.float32).reshape(S, H)
    w_proj = np.asarray(w_proj, dtype=np.float32)
    w_o = np.asarray(w_o, dtype=np.float32)

    # BitNet b1.58 per-tensor absmean quantization (ternary, scale factored out)
    s_p = np.float32(np.mean(np.abs(w_proj), dtype=np.float32)) + np.float32(1e-5)
    s_o = np.float32(np.mean(np.abs(w_o), dtype=np.float32)) + np.float32(1e-5)
    tp = np.clip(np.round(w_proj / s_p), -1.0, 1.0).astype(np.float32)
    to = np.clip(np.round(w_o / s_o), -1.0, 1.0).astype(np.float32)

    # x split into fp8 hi + residual: hi+res matmul chains recover ~bf16
    # accuracy while running at fp8 DoubleRow rates
    xt32 = np.ascontiguousarray(x.T)                            # [H, S] f32
    xh = xt32.astype(ml_dtypes.float8_e4m3)
    xr = (xt32 - xh.astype(np.float32)).astype(ml_dtypes.float8_e4m3)

    # RoPE tables, feature-major, rotate-half sign folded into sin
    inv_freq = (1.0 / (ROPE_BASE ** (np.arange(0, D, 2, dtype=np.float32) / D))
                ).astype(np.float32)
    t = np.arange(S, dtype=np.float32)
    freqs = np.outer(inv_freq, t).astype(np.float32)    # [64, S]
    cosT = np.concatenate([np.cos(freqs), np.cos(freqs)], 0).astype(ml_dtypes.bfloat16)
    # halves swapped vs the math ([-sin | sin]): rope reads the half at the
    # partition base of its OTHER operand (q8[64:] pairs with sins[64:])
    sinS = np.concatenate([np.sin(freqs), -np.sin(freqs)], 0).astype(ml_dtypes.bfloat16)

    # shifted tril mask bank: tri[p, x] = 1 if p <= x - 384
    p = np.arange(128)[:, None]
    xx = np.arange(896)[None, :]
    tri = (p <= xx - 384).astype(ml_dtypes.bfloat16)

    esc = np.full((128, 1), s_p * s_p / np.sqrt(np.float32(D)), np.float32)
    osc = np.full((128, 1), s_p * s_o, np.float32)

    in_maps = []
    for c in range(NCORES):
        r = slice(c * HPC * D, (c + 1) * HPC * D)       # 256 features
        wt_c = np.ascontiguousarray(
            np.concatenate([tp[:H][r], tp[H:2 * H][r], tp[2 * H:][r]], 0).T
        ).astype(ml_dtypes.float8_e4m3)
        wot_c = np.ascontiguousarray(to[:, r].T).astype(ml_dtypes.float8_e4m3)
        in_maps.append({
            "xh": xh, "xr": xr, "wt": wt_c, "wot": wot_c, "cost": cosT,
            "sins": sinS, "tri": tri,
            "osq": np.ones((128, 128), ml_dtypes.bfloat16),
            "esc": esc, "osc": osc,
        })
    return in_maps


def kernel(hidden_states, attention_mask, w_proj, w_o):
    global _built
    if _built is None:
        _built = _build()
    nc = _built
    in_maps = _host_prep(hidden_states, w_proj, w_o)
    res = run_bass_kernel_spmd(nc, in_maps, core_ids=list(range(NCORES)))
    acc = np.zeros((S, H), np.float32)
    for c in range(NCORES):
        acc += res.results[c]["out"].astype(np.float32)
    return acc.reshape(1, S, H)

